# revision 4
# baseline (speedup 1.0000x reference)
"""AttentionAggregator kernel for 8 trn2 NeuronCores.

Math (exploits linearity of the shared feat_weights matmul):
  wa = feat_weights @ attn_weights                      # [128,1]
  logit[n,k]  = neigh[n,k,:]@wa + self[n,:]@wa
  E[n,k]      = exp(leaky_relu(logit[n,k], 0.2))
  agg[n,:]    = sum_k E[n,k] * neigh[n,k,:]             # unnormalized
  out[n,:]    = relu((self[n,:] + agg[n,:]/sum_k E[n,k]) @ W + bias)

Data-parallel over nodes: 50000 padded to 50176 = 8 cores * 49 tiles * 128.
"""

import sys

sys.path.insert(0, "/opt/trn_rl_repo")

import numpy as np
import ml_dtypes

import concourse.bass as bass
import concourse.bacc as bacc
import concourse.mybir as mybir
import concourse.tile as tile
from concourse.bass_utils import run_bass_kernel_spmd

N_CORES = 8
D = 128
K = 32
P = 128
TILES = 49                       # node tiles per core
NODES_PC = TILES * P             # 6272 nodes per core
ROWS_PC = NODES_PC * K           # 200704 neigh rows per core
N_FULL = 50000

F32 = mybir.dt.float32
BF16 = mybir.dt.bfloat16
BF = ml_dtypes.bfloat16

_cache = {}


def _build():
    nc = bacc.Bacc("TRN2", target_bir_lowering=False, debug=False)

    self_t = nc.dram_tensor("self_sh", [NODES_PC, D], F32, kind="ExternalInput")
    neigh_t = nc.dram_tensor("neigh_sh", [ROWS_PC, D], F32, kind="ExternalInput")
    w_t = nc.dram_tensor("w_bf", [D, D], BF16, kind="ExternalInput")
    wa_t = nc.dram_tensor("wa_bf", [D, 1], BF16, kind="ExternalInput")
    warep_t = nc.dram_tensor("wa_rep", [P, D], BF16, kind="ExternalInput")
    ident_t = nc.dram_tensor("ident_bf", [P, P], BF16, kind="ExternalInput")
    ones_t = nc.dram_tensor("ones_bf", [1, P], BF16, kind="ExternalInput")
    bias_t = nc.dram_tensor("bias_bf", [1, D], BF16, kind="ExternalInput")
    out_t = nc.dram_tensor("out", [NODES_PC, D], F32, kind="ExternalOutput")

    GR = 8                        # k's per transpose PSUM group
    NG = K // GR                  # 4 groups

    with tile.TileContext(nc) as tc:
        with (
            tc.tile_pool(name="const", bufs=1) as cpool,
            tc.tile_pool(name="big", bufs=1) as bigpool,
            tc.tile_pool(name="nb", bufs=3) as nbpool,
            tc.tile_pool(name="work", bufs=2) as wpool,
            tc.tile_pool(name="small", bufs=2) as smpool,
            tc.tile_pool(name="ps_t", bufs=2, space="PSUM") as ps_t,
            tc.tile_pool(name="ps_misc", bufs=1, space="PSUM") as ps_misc,
        ):
            # ---- constants ----
            ident = cpool.tile([P, P], BF16)
            w_sb = cpool.tile([D, D], BF16)
            wa_sb = cpool.tile([D, 1], BF16)
            warep = cpool.tile([P, D], BF16)
            ones_sb = cpool.tile([1, P], BF16)
            bias_sb = cpool.tile([1, D], BF16)
            nc.sync.dma_start(ident[:], ident_t[:])
            nc.sync.dma_start(w_sb[:], w_t[:])
            nc.sync.dma_start(wa_sb[:], wa_t[:])
            nc.sync.dma_start(warep[:], warep_t[:])
            nc.sync.dma_start(ones_sb[:], ones_t[:])
            nc.sync.dma_start(bias_sb[:], bias_t[:])

            # ---- whole-core self vecs (bf16) stay in SBUF ----
            self_sb = bigpool.tile([P, TILES * D], BF16)
            for t in range(TILES):
                nc.gpsimd.dma_start(
                    self_sb[:, t * D : (t + 1) * D],
                    self_t[t * P : (t + 1) * P, :],
                )

            for t in range(TILES):
                # ---- load neigh tile: partition = node-local, free = (k, d) ----
                nb = nbpool.tile([P, K * D], BF16, tag="nb")
                nc.gpsimd.dma_start(
                    nb[:],
                    neigh_t[t * P * K : (t + 1) * P * K, :].rearrange(
                        "(p c) d -> p (c d)", p=P
                    ),
                )
                sf = self_sb[:, t * D : (t + 1) * D]

                # ---- self logit: sl[n] = sum_d self[n,d]*wa[d] ----
                junk = smpool.tile([P, D], BF16, tag="junk")
                sl = smpool.tile([P, 1], F32, tag="sl")
                nc.vector.scalar_tensor_tensor(
                    junk[:], sf, 1.0, warep[:],
                    mybir.AluOpType.mult, mybir.AluOpType.mult,
                    accum_out=sl[:],
                )

                # ---- transpose k-slices on PE; copy back as bf16 ----
                t_sb = wpool.tile([P, K * D], BF16, tag="t_sb")
                for g in range(NG):
                    t_ps = ps_t.tile([P, GR * D], F32, tag="t_ps")
                    for c in range(GR):
                        k = g * GR + c
                        nc.tensor.matmul(
                            t_ps[:, c * D : (c + 1) * D],
                            nb[:, k * D : (k + 1) * D],
                            ident[:],
                        )
                    dst = t_sb[:, g * GR * D : (g + 1) * GR * D]
                    if g % 2 == 0:
                        nc.vector.tensor_copy(dst, t_ps[:])
                    else:
                        nc.scalar.copy(dst, t_ps[:])

                # ---- logits matvec: lhsT = T_k -> logits[node, k] ----
                log_ps = ps_misc.tile([P, K], F32, tag="log_ps")
                for k in range(K):
                    nc.tensor.matmul(
                        log_ps[:, k : k + 1],
                        t_sb[:, k * D : (k + 1) * D],
                        wa_sb[:],
                    )

                # ---- leaky_relu(logits + sl) on DVE, exp on ACT ----
                a_sb = smpool.tile([P, K], F32, tag="a_sb")
                nc.vector.tensor_scalar_add(a_sb[:], log_ps[:], sl[:])
                l_sb = smpool.tile([P, K], F32, tag="l_sb")
                nc.vector.scalar_tensor_tensor(
                    l_sb[:], a_sb[:], 0.2, a_sb[:],
                    mybir.AluOpType.mult, mybir.AluOpType.max,
                )
                e_sb = smpool.tile([P, K], F32, tag="e_sb")
                nc.scalar.activation(e_sb[:], l_sb[:], mybir.ActivationFunctionType.Exp)

                # ---- softmax denom + reciprocal ----
                s_sb = smpool.tile([P, 1], F32, tag="s_sb")
                nc.vector.tensor_reduce(
                    s_sb[:], e_sb[:], axis=mybir.AxisListType.X, op=mybir.AluOpType.add
                )
                r_sb = smpool.tile([P, 1], F32, tag="r_sb")
                nc.vector.reciprocal(r_sb[:], s_sb[:])

                # ---- diag(E[:,k]) tiles, then PE combine ----
                diag = wpool.tile([P, K * D], BF16, tag="diag")
                for k in range(K):
                    nc.vector.tensor_scalar_mul(
                        diag[:, k * D : (k + 1) * D], ident[:], e_sb[:, k : k + 1]
                    )
                agg_ps = ps_misc.tile([P, D], F32, tag="agg_ps")
                for k in range(K):
                    nc.tensor.matmul(
                        agg_ps[:],
                        diag[:, k * D : (k + 1) * D],
                        nb[:, k * D : (k + 1) * D],
                        start=(k == 0),
                        stop=(k == K - 1),
                    )

                # ---- Sn = self + R*agg ; transpose; final matmul + bias; relu ----
                sn_sb = smpool.tile([P, D], BF16, tag="sn_sb")
                nc.vector.scalar_tensor_tensor(
                    sn_sb[:], agg_ps[:], r_sb[:], sf,
                    mybir.AluOpType.mult, mybir.AluOpType.add,
                )
                snt_ps = ps_misc.tile([P, D], F32, tag="snt_ps")
                nc.tensor.matmul(snt_ps[:], sn_sb[:], ident[:])
                snt_sb = smpool.tile([P, D], BF16, tag="snt_sb")
                nc.scalar.copy(snt_sb[:], snt_ps[:])

                o_ps = ps_misc.tile([P, D], F32, tag="o_ps")
                nc.tensor.matmul(o_ps[:], ones_sb[:], bias_sb[:], start=True, stop=False)
                nc.tensor.matmul(o_ps[:], snt_sb[:], w_sb[:], start=False, stop=True)
                o_sb = smpool.tile([P, D], F32, tag="o_sb")
                nc.scalar.activation(
                    o_sb[:], o_ps[:], mybir.ActivationFunctionType.Relu
                )
                nc.sync.dma_start(out_t[t * P : (t + 1) * P, :], o_sb[:])

    nc.compile()
    return nc


def kernel(self_vecs, neigh_vecs, feat_weights, attn_weights, bias, num_neighbors):
    self_vecs = np.asarray(self_vecs, dtype=np.float32)
    neigh_vecs = np.asarray(neigh_vecs, dtype=np.float32)
    feat_weights = np.asarray(feat_weights, dtype=np.float32)
    attn_weights = np.asarray(attn_weights, dtype=np.float32)
    bias = np.asarray(bias, dtype=np.float32)

    n = self_vecs.shape[0]
    n_pad = N_CORES * NODES_PC
    self_p = np.zeros((n_pad, D), np.float32)
    self_p[:n] = self_vecs
    neigh_p = np.zeros((n_pad * K, D), np.float32)
    neigh_p[: n * K] = neigh_vecs

    wa = (feat_weights @ attn_weights).astype(np.float32)        # [128,1]
    w_bf = feat_weights.astype(BF)
    wa_bf = wa.astype(BF)
    wa_rep = np.tile(wa.reshape(1, D), (P, 1)).astype(BF)
    ident = np.eye(P, dtype=np.float32).astype(BF)
    ones_bf = np.ones((1, P), np.float32).astype(BF)
    bias_bf = bias.reshape(1, D).astype(BF)

    if "nc" not in _cache:
        _cache["nc"] = _build()
    nc = _cache["nc"]

    in_maps = []
    for c in range(N_CORES):
        in_maps.append(
            {
                "self_sh": self_p[c * NODES_PC : (c + 1) * NODES_PC],
                "neigh_sh": neigh_p[c * ROWS_PC : (c + 1) * ROWS_PC],
                "w_bf": w_bf,
                "wa_bf": wa_bf,
                "wa_rep": wa_rep,
                "ident_bf": ident,
                "ones_bf": ones_bf,
                "bias_bf": bias_bf,
            }
        )

    import os

    trace = os.environ.get("KERNEL_TRACE") == "1"
    res = run_bass_kernel_spmd(nc, in_maps, list(range(N_CORES)), trace=trace)
    _cache["last_result"] = res
    out = np.concatenate([res.results[c]["out"] for c in range(N_CORES)], axis=0)
    return out[:n].astype(np.float32)
